# revision 37
# baseline (speedup 1.0000x reference)
"""BertSelfAttention fused kernel for Trainium2, 8 NeuronCores.

Sharding: tensor-parallel over heads. 16 heads / 8 cores = 2 heads per core.
Core c owns heads 2c, 2c+1 == output feature columns [128c, 128c+128).
Every core reads the full hidden_states (pre-transposed on host to [D, B*S])
plus its 128-column slice of Wq/Wk/Wv (pre-transposed to [D, 128]); it writes
its [B*S, 128] slab of the output. No cross-core communication.

Per-core device program (B=4 batches, S=2048, D=1024, HD=64):
  stage 0: load weights/biases/mask constants; f = exp(mask) per key.
  per batch b:
    stage 1 (projection): QT,KT [128, 2048] (partition = head-elem dim,
      2 heads stacked), V [128 tok, 16 kblk, 130] where cols 0:64 head A,
      64 = denom col, 65:129 head B, 129 = denom col; V rows scaled by
      f=exp(mask) (folds the additive attention mask into the softmax
      weights exactly) and the denom cols set to f, so the softmax
      denominator comes out of the PV matmul for free.
    stage 2 (attention), per 512-query group, software-pipelined:
      for each 128-key block: S^T = K^T.T @ Q^T  (PSUM, keys on
        partitions; the two heads run as concurrent PE row-tiles)
        E = exp(S^T / 8): key blocks in DVE_KB run on the Vector engine
        (int32-cast Schraudolph seed + custom quadratic bit-correction
        op, see EXP_CORR), the rest on the Scalar engine's exp LUT —
        the two engines share what is otherwise the serial exp wall.
        ctx~ += [V|f].T @ E  (PSUM accumulate, [65, 512] per head;
                              row 64 = softmax denominator)
      epilogue: PE-transpose ctx~ 128-query chunks, out = num/den (DVE),
        DMA [128, 128] (2 heads) to the output slab.
    Scheduling: scores(i+1) is emitted before exp(i)/pv(i) including
    across query-group and batch boundaries, and the next batch's
    projection chains are emitted inside the attention loop (aligned
    with the DVE key blocks' longer exp latency) so ready projection
    matmuls fill the PE bubbles where attention waits on exp.

Matmul operands are fp16 (1 PE cycle/column vs 2 for fp32r tf32 and 4
for fp32; fp16's 10 mantissa bits keep the output within ~5e-4 of the
fp32 reference; all tensors here fit fp16 range). PSUM accumulation is
always fp32.
"""

import sys

sys.path.insert(0, "/opt/trn_rl_repo")

from contextlib import ExitStack

import numpy as np

import concourse.bass as bass
import concourse.dve_ops as dve_ops
import concourse.mybir as mybir
import concourse.tile as tile
from concourse import bacc
from concourse.bass import ds
from concourse.dve_spec import (
    C0, C1, C2, C3, Bin, One, Spec, Src0, _has_src1, _spill_c3_to_src1, lower,
)
from concourse.dve_uop import AluOp, DveOpSpec
from concourse.masks import make_identity

B, S, D = 4, 2048, 1024
H, HD = 16, 64
NCORES = 8
CW = 128  # output columns per core (2 heads * 64)
P = 128

FP32 = mybir.dt.float32
FP32R = mybir.dt.float32r
BF16 = mybir.dt.bfloat16
FP16 = mybir.dt.float16

# matmul-operand dtype: "fp32" (exact, 4 cyc/col), "fp32r" (tf32, 2 cyc/col),
# "bf16"/"fp16" (1 cyc/col; fp16 carries 10 mantissa bits vs bf16's 8 and all
# tensors here fit fp16 range), "mixed" (projections fp32r, attention fp16)
MM_DTYPE = "fp16"

# Per query group, key blocks whose softmax exp runs on the Vector engine
# (2-instruction Schraudolph: int32-cast seed + quadratic bit-field
# correction, ~3.5e-3 max rel err) instead of the Scalar engine's exp LUT.
# ACT's exp throughput (1 elem/lane/cycle @ 1.2 GHz, 33.5M elems/core) is
# the serial wall of the attention loop; splitting with DVE removes it.
DVE_KB = (2, 7, 12)

I32 = mybir.dt.int32
# seed: i = int32(score * (2^23*log2e/8) + 127*2^23); /8 folds the 1/sqrt(HD)
# softmax scale. bits(i) as fp32 = 2^k*(1+u) ~ exp(score/8).
SEED_S = float(np.float32(2.0**23) * np.float32(1.4426950408889634) / 8.0)
SEED_B = float(np.float32(127 * 2**23))
# correction: out = y * (Q0 + v*(Q1 + v*Q2)), v = 1+u via bit mask
EXP_Q0 = 1.4569739756811277
EXP_Q1 = -0.6941217487887653
EXP_Q2 = 0.23368320766312967
MASK_F = float(np.array([0x007FFFFF], np.int32).view(np.float32)[0])


def _exp_corr_ref(in0, in1, s0, s1, imm2):
    bits = np.ascontiguousarray(np.asarray(in0, np.float32)).view(np.int32)
    m = (np.float32(s0).view(np.int32) if np.ndim(s0) == 0 else
         np.asarray(s0, np.float32).reshape(-1, 1).view(np.int32))
    v = ((bits & m) | 0x3F800000).view(np.float32)
    c2 = np.float32(np.asarray(in1, np.float32).reshape(-1, 1))
    p = np.float32(s1) + v * (np.float32(imm2) + v * c2)
    return (np.asarray(in0, np.float32) * p).astype(np.float32)


def _make_exp_corr_op():
    name = "EXP_CORR_ANT"
    for o in dve_ops.OPS:
        if o.name == name:
            return o
    u = Bin(AluOp.BITWISE_AND, Src0, C0)   # C0 = mantissa mask 0x007FFFFF
    v = Bin(AluOp.BITWISE_OR, u, One)      # 1+u in [1,2)
    body = _spill_c3_to_src1(Src0 * (C1 + v * (C2 + v * C3)))
    spec = Spec(body=body, reference=_exp_corr_ref)
    row = dve_ops._CUSTOM_DVE_ROW_BASE + len(dve_ops.OPS)
    shas = {}
    for ver in ("v3", "v4"):
        shas[ver] = DveOpSpec(
            name=name, opcode=row, uops=lower(spec, ver=ver),
            rd1_en=_has_src1(spec)).sha(ver)
    op = dve_ops.DveOp(name, spec, subdim=False, uops_sha=shas)
    dve_ops.OPS.append(op)
    dve_ops.CUSTOM_DVE_SPECS[name] = spec
    dve_ops._SUB_OPCODE_FOR_NAME[name] = row
    return op


EXP_CORR = _make_exp_corr_op()


def _add_scale_ref(in0, in1, s0, s1, imm2):
    s = np.asarray(s0, np.float32).reshape(-1, 1) if np.ndim(s0) else s0
    return ((np.asarray(in0, np.float32) + np.asarray(in1, np.float32))
            * np.float32(s)).astype(np.float32)


def _make_add_scale_op():
    name = "ADD_SCALE_ANT"
    for o in dve_ops.OPS:
        if o.name == name:
            return o
    from concourse.dve_spec import Src1
    spec = Spec(body=(Src0 + Src1) * C0, reference=_add_scale_ref)
    row = dve_ops._CUSTOM_DVE_ROW_BASE + len(dve_ops.OPS)
    shas = {}
    for ver in ("v3", "v4"):
        shas[ver] = DveOpSpec(
            name=name, opcode=row, uops=lower(spec, ver=ver),
            rd1_en=_has_src1(spec)).sha(ver)
    op = dve_ops.DveOp(name, spec, subdim=False, uops_sha=shas)
    dve_ops.OPS.append(op)
    dve_ops.CUSTOM_DVE_SPECS[name] = spec
    dve_ops._SUB_OPCODE_FOR_NAME[name] = row
    return op


ADD_SCALE = _make_add_scale_op()


def _mm_dts(mm_dtype):
    """-> (projection operand dtype, attention operand dtype)"""
    if mm_dtype == "mixed":
        return FP32R, FP16
    dt = {"fp32": FP32, "fp32r": FP32R, "bf16": BF16, "fp16": FP16}[mm_dtype]
    return dt, dt


def emit_kernel(ctx: ExitStack, tc: tile.TileContext, aps: dict, b_sz: int,
                s_sz: int, mm_dtype: str):
    nc = tc.nc
    n_tok = b_sz * s_sz
    TB = min(512, s_sz)           # projection token-block / query-group size
    n_tb = s_sz // TB             # token blocks per batch
    n_kb = s_sz // P              # key blocks per batch
    n_qg = s_sz // TB             # query groups per batch
    DCH = D // P                  # contraction chunks (8)
    n_bk = b_sz * n_kb            # total key blocks

    PJ, AT = _mm_dts(mm_dtype)
    hid_t, wqt, wkt, wvt, bq, bk, bv, mask, out = (
        aps["hidden_t"], aps["wqt"], aps["wkt"], aps["wvt"], aps["bq"],
        aps["bk"], aps["bv"], aps["mask"], aps["out"])

    const = ctx.enter_context(tc.tile_pool(name="const", bufs=1))
    hidp = ctx.enter_context(tc.tile_pool(name="hidp", bufs=4))
    qkv = ctx.enter_context(tc.tile_pool(name="qkv", bufs=4))
    epool = ctx.enter_context(tc.tile_pool(name="epool", bufs=6))
    i32p = ctx.enter_context(tc.tile_pool(name="i32p", bufs=2))
    csb = ctx.enter_context(tc.tile_pool(name="csb", bufs=3))
    ostage = ctx.enter_context(tc.tile_pool(name="ostage", bufs=4))
    small = ctx.enter_context(tc.tile_pool(name="small", bufs=8))
    vtmpp = ctx.enter_context(tc.tile_pool(name="vtmpp", bufs=2))
    psA = ctx.enter_context(tc.tile_pool(name="psA", bufs=2, space="PSUM"))
    psC = ctx.enter_context(tc.tile_pool(name="psC", bufs=1, space="PSUM"))
    psP = ctx.enter_context(tc.tile_pool(name="psP", bufs=2, space="PSUM"))

    # ---- stage 0: constants ----
    # Emission order tuned for startup: the first projection chain needs
    # wq + hid(0), so those DMAs go first on the queue; the identity (which
    # gates PE's first instruction, the mask transpose) is built on gpsimd
    # before the bv-broadcast DMA is queued there.
    wq_sb = const.tile([P, DCH, CW], PJ)
    nc.sync.dma_start(wq_sb, wqt.rearrange("(c p) m -> p c m", p=P))
    bq_sb = const.tile([P, 1], FP32)
    nc.sync.dma_start(bq_sb, bq.rearrange("(p o) -> p o", o=1))
    mask_bo = const.tile([n_bk, P], FP32)
    nc.sync.dma_start(mask_bo, mask.rearrange("b (o p) -> (b o) p", p=P))

    # [P,1] tile carrying the spilled quadratic coefficient for EXP_CORR
    q2t = const.tile([P, 1], FP32)
    nc.vector.memset(q2t, EXP_Q2)

    ident = const.tile([P, P], FP32)
    f_sb = const.tile([P, n_bk], FP32)

    def emit_mask_setup():
        # Emitted after the first projection token-block so the identity
        # build (gpsimd) and mask DMA/transpose don't gate the PE's first
        # projection matmuls. f = exp(mask) is first consumed by the DVE
        # V-scaling, long after the first proj chain.
        make_identity(nc, ident)
        mask_ps = psP.tile([P, n_bk], FP32, tag="proj", name="mask_ps")
        nc.tensor.matmul(mask_ps, mask_bo, ident[:n_bk, :n_bk],
                         is_transpose=True)
        nc.scalar.activation(f_sb, mask_ps, mybir.ActivationFunctionType.Exp)

    wk_sb = const.tile([P, DCH, CW], PJ)
    nc.sync.dma_start(wk_sb, wkt.rearrange("(c p) m -> p c m", p=P))
    wv_sb = const.tile([P, DCH, CW], PJ)
    nc.sync.dma_start(wv_sb, wvt.rearrange("(c p) m -> p c m", p=P))
    bk_sb = const.tile([P, 1], FP32)
    nc.sync.dma_start(bk_sb, bk.rearrange("(p o) -> p o", o=1))
    # bv broadcast to all partitions: [128, 128], every row = bv
    bvb = const.tile([P, CW], FP32)
    nc.gpsimd.dma_start(
        out=bvb,
        in_=bass.AP(tensor=bv.tensor, offset=bv.offset, ap=[[0, P], bv.ap[0]]),
    )

    qkv_tiles: dict = {}
    hid_tiles: dict = {}

    def emit_proj_dma(b, tb):
        """Allocate batch tiles + issue the hidden-state DMA for one token
        block; the matmul chains follow via emit_proj_part."""
        if tb == 0:
            qkv_tiles[b] = (
                qkv.tile([P, s_sz], AT, tag="qt", name="qt_b"),
                qkv.tile([P, s_sz], AT, tag="kt", name="kt_b"),
                qkv.tile([P, n_kb, 130], AT, tag="v", name="v_b"),
            )
        tok0 = b * s_sz + tb * TB
        hid_tile = hidp.tile([P, DCH, TB], PJ, tag="hid", name="hid_tile")
        hid_src = hid_t.rearrange("(c p) n -> p c n", p=P)[:, :, ds(tok0, TB)]
        nc.sync.dma_start(hid_tile[:, 0:DCH // 2], hid_src[:, 0:DCH // 2])
        nc.sync.dma_start(hid_tile[:, DCH // 2:DCH],
                          hid_src[:, DCH // 2:DCH])
        hid_tiles[(b, tb)] = hid_tile

    def emit_proj_part(b, tb, part, defer_bias=False):
        """One self-contained projection matmul chain (~0.5-2.1us of PE
        work). Scattered between attention kb iterations so these
        ready-to-run matmuls fill the PE bubbles where attention waits on
        exp results."""
        qt_b, kt_b, v_b = qkv_tiles[b]
        hid_tile = hid_tiles[(b, tb)]
        if part in ("pq", "pk"):
            w_sb, bias, dst = ((wq_sb, bq_sb, qt_b) if part == "pq" else
                               (wk_sb, bk_sb, kt_b))
            ps = psP.tile([P, TB], FP32, tag="proj", name="ps")
            for c in range(DCH):
                nc.tensor.matmul(ps, w_sb[:, c, :],
                                 hid_tile[:, c, :],
                                 start=(c == 0), stop=(c == DCH - 1))
            if part == "pk" and defer_bias:
                # deferred: flushed right after the epilogue ctx copies so
                # those aren't queued behind this add on the DVE FIFO (kt
                # isn't read until the next batch's attention). Must flush
                # before the tp transposes allocate from the same psP
                # rotation.
                pending_bias.append((dst[:, ds(tb * TB, TB)], ps, bias))
            else:
                nc.vector.tensor_scalar_add(dst[:, ds(tb * TB, TB)], ps,
                                            bias)
            return
        s4_range = (0, 1) if part == "pv01" else (2, 3)
        for s4 in s4_range:
            kbg = tb * (TB // P) + s4  # key block index within batch
            pv = psP.tile([P, CW], FP32, tag="proj", name="pv")
            for c in range(DCH):
                nc.tensor.matmul(
                    pv, hid_tile[:, c, ds(s4 * P, P)],
                    wv_sb[:, c, :],
                    start=(c == 0), stop=(c == DCH - 1))
            vtmp = vtmpp.tile([P, CW], FP32, tag="vtmp", name="vtmp")
            nc.vector.tensor_add(vtmp, pv, bvb)
            fcol = f_sb[:, ds(b * n_kb + kbg, 1)]
            nc.vector.tensor_scalar_mul(v_b[:, kbg, 0:HD], vtmp[:, 0:HD],
                                        fcol)
            nc.vector.tensor_scalar_mul(v_b[:, kbg, 65:129],
                                        vtmp[:, HD:CW], fcol)
            nc.vector.tensor_copy(v_b[:, kbg, ds(HD, 1)], fcol)
            nc.vector.tensor_copy(v_b[:, kbg, ds(129, 1)], fcol)

    PROJ_PARTS = ("pv01", "pv23", "pq", "pk")
    pending_bias: list = []

    emit_mask_setup()
    for tb in range(n_tb):
        emit_proj_dma(0, tb)
        for part in PROJ_PARTS:
            emit_proj_part(0, tb, part)

    def emit_scores(b, qg, kb):
        qt_b, kt_b, _ = qkv_tiles[b]
        q0 = qg * TB
        st = psA.tile([P, 2 * TB], FP32, tag="st", name="st")
        nc.tensor.matmul(st[:, 0:TB],
                         kt_b[0:HD, ds(kb * P, P)],
                         qt_b[0:HD, ds(q0, TB)],
                         start=True, stop=True)
        nc.tensor.matmul(st[:, ds(TB, TB)],
                         kt_b[HD:P, ds(kb * P, P)],
                         qt_b[HD:P, ds(q0, TB)],
                         start=True, stop=True)
        return st

    # ---- stage 2: attention, software-pipelined ----
    # scores(i+1) is emitted before exp(i) and pv(i), including across
    # query-group and batch boundaries, so the PE always has the next
    # scores pair queued while exp runs. exp of each key block goes to ACT
    # (LUT) or DVE (Schraudolph seed + quadratic correction) per DVE_KB so
    # the two engines share the softmax exp wall; projection chains for the
    # next batch are emitted between exp and pv of the DVE key blocks,
    # where the PE otherwise idles for the (longer) DVE exp latency.
    groups = [(b, qg) for b in range(b_sz) for qg in range(n_qg)]
    PROJ_AT = (2, 7, 12, 14)
    st_cur = emit_scores(0, 0, 0)
    for gi, (b, qg) in enumerate(groups):
        q0 = qg * TB
        _, _, v_b = qkv_tiles[b]
        ctx_ps = psC.tile([P, 2 * TB], FP32, tag="ctx", name="ctx_ps")

        for kb in range(n_kb):
            if kb + 1 < n_kb:
                st_next = emit_scores(b, qg, kb + 1)
            elif gi + 1 < len(groups):
                st_next = emit_scores(*groups[gi + 1], 0)
            else:
                st_next = None
            if kb == 0 and b + 1 < b_sz:
                emit_proj_dma(b + 1, qg)
            e_t = epool.tile([P, 2 * TB], AT, tag="e", name="e_t")
            if kb in DVE_KB:
                i32 = i32p.tile([P, 2 * TB], I32, tag="i32", name="i32")
                for h in range(2):
                    sl = ds(h * TB, TB)
                    nc.vector.tensor_scalar(i32[:, sl], st_cur[:, sl],
                                            SEED_S, SEED_B,
                                            mybir.AluOpType.mult,
                                            mybir.AluOpType.add)
                    nc.vector._custom_dve(EXP_CORR, out=e_t[:, sl],
                                          in0=i32[:, sl].bitcast(FP32),
                                          in1=q2t, s0=MASK_F, s1=EXP_Q0,
                                          imm2=EXP_Q1)
            else:
                nc.scalar.activation(e_t, st_cur,
                                     mybir.ActivationFunctionType.Exp,
                                     scale=1.0 / 8.0)
            if kb in PROJ_AT and b + 1 < b_sz:
                emit_proj_part(b + 1, qg, PROJ_PARTS[PROJ_AT.index(kb)],
                               defer_bias=True)
            nc.tensor.matmul(ctx_ps[0:65, 0:TB],
                             v_b[:, kb, 0:65],
                             e_t[:, 0:TB],
                             start=(kb == 0), stop=(kb == n_kb - 1))
            nc.tensor.matmul(ctx_ps[0:65, ds(TB, TB)],
                             v_b[:, kb, ds(65, 65)],
                             e_t[:, ds(TB, TB)],
                             start=(kb == 0), stop=(kb == n_kb - 1))
            st_cur = st_next

        ctx_sbs = [csb.tile([65, TB], FP32, tag="csb", name="ctx_sb")
                   for _ in range(2)]
        for h4 in range(2):
            for j in range(2):
                nc.vector.tensor_copy(
                    ctx_sbs[j][:, ds(h4 * (TB // 2), TB // 2)],
                    ctx_ps[0:65, ds(j * TB + h4 * (TB // 2), TB // 2)])
        while pending_bias:
            dst_sl, ps_t, bias_t = pending_bias.pop(0)
            nc.vector.tensor_scalar_add(dst_sl, ps_t, bias_t)
        for sub in range(TB // P):
            ost = ostage.tile([P, CW], FP32, tag="ost", name="ost")
            for j in range(2):
                tp = psP.tile([P, 65], FP32, tag="proj", name="tp")
                nc.tensor.matmul(tp, ctx_sbs[j][:, ds(sub * P, P)],
                                 ident[0:65, 0:65], is_transpose=True)
                rcp = small.tile([P, 1], FP32, tag="rcp", name="rcp")
                nc.vector.reciprocal_approx_fast(rcp, tp[:, ds(HD, 1)])
                nc.vector.tensor_scalar_mul(ost[:, ds(j * HD, HD)],
                                            tp[:, 0:HD], rcp)
            tok0 = b * s_sz + q0 + sub * P
            nc.sync.dma_start(out[ds(tok0, P), :], ost)


def build_program(b_sz=B, s_sz=S, mm_dtype=None):
    if mm_dtype is None:
        mm_dtype = MM_DTYPE
    PJ, _ = _mm_dts(mm_dtype)
    nc = bacc.Bacc("TRN2", target_bir_lowering=False, debug=False)
    n_tok = b_sz * s_sz
    aps = {
        "hidden_t": nc.dram_tensor("hidden_t", [D, n_tok], PJ,
                                   kind="ExternalInput").ap(),
        "wqt": nc.dram_tensor("wqt", [D, CW], PJ, kind="ExternalInput").ap(),
        "wkt": nc.dram_tensor("wkt", [D, CW], PJ, kind="ExternalInput").ap(),
        "wvt": nc.dram_tensor("wvt", [D, CW], PJ, kind="ExternalInput").ap(),
        "bq": nc.dram_tensor("bq", [CW], FP32, kind="ExternalInput").ap(),
        "bk": nc.dram_tensor("bk", [CW], FP32, kind="ExternalInput").ap(),
        "bv": nc.dram_tensor("bv", [CW], FP32, kind="ExternalInput").ap(),
        "mask": nc.dram_tensor("mask", [b_sz, s_sz], FP32,
                               kind="ExternalInput").ap(),
        "out": nc.dram_tensor("out", [n_tok, CW], FP32,
                              kind="ExternalOutput").ap(),
    }
    with tile.TileContext(nc) as tc:
        with ExitStack() as ctx:
            emit_kernel(ctx, tc, aps, b_sz, s_sz, mm_dtype)
    nc.compile()
    return nc


def make_in_maps(hidden_states, attention_mask, Wq, bq, Wk, bk, Wv, bv,
                 b_sz=B, s_sz=S):
    hidden_states = np.asarray(hidden_states, dtype=np.float32)
    x = hidden_states.reshape(b_sz * s_sz, D)
    hid_t = np.ascontiguousarray(x.T)
    mask = np.ascontiguousarray(
        np.broadcast_to(
            np.asarray(attention_mask, dtype=np.float32).reshape(b_sz, 1, 1,
                                                                 s_sz),
            (b_sz, 1, 1, s_sz)).reshape(b_sz, s_sz))
    Wq, Wk, Wv = (np.asarray(w, dtype=np.float32) for w in (Wq, Wk, Wv))
    bq, bk, bv = (np.asarray(v, dtype=np.float32) for v in (bq, bk, bv))
    # hidden/weights are pre-cast on the host to the projection matmul dtype
    # (same round-to-nearest the device casting DMA would apply)
    pj_np = mybir.dt.np(_mm_dts(MM_DTYPE)[0])
    hid_t = hid_t.astype(pj_np)
    in_maps = []
    for c in range(NCORES):
        rows = slice(c * CW, (c + 1) * CW)
        in_maps.append({
            "hidden_t": hid_t,
            "wqt": np.ascontiguousarray(Wq[rows, :].T).astype(pj_np),
            "wkt": np.ascontiguousarray(Wk[rows, :].T).astype(pj_np),
            "wvt": np.ascontiguousarray(Wv[rows, :].T).astype(pj_np),
            "bq": np.ascontiguousarray(bq[rows]),
            "bk": np.ascontiguousarray(bk[rows]),
            "bv": np.ascontiguousarray(bv[rows]),
            "mask": mask,
        })
    return in_maps


_NC_CACHE = {}


def _get_program():
    key = (B, S, MM_DTYPE)
    if key not in _NC_CACHE:
        _NC_CACHE[key] = build_program(B, S, MM_DTYPE)
    return _NC_CACHE[key]


def kernel(hidden_states, attention_mask, Wq, bq, Wk, bk, Wv, bv):
    from concourse.bass_utils import run_bass_kernel_spmd

    nc = _get_program()
    in_maps = make_in_maps(hidden_states, attention_mask, Wq, bq, Wk, bk, Wv,
                           bv)
    res = run_bass_kernel_spmd(nc, in_maps, list(range(NCORES)))
    out = np.empty((B, S, D), dtype=np.float32)
    for c in range(NCORES):
        out[:, :, c * CW:(c + 1) * CW] = res.results[c]["out"].reshape(
            B, S, CW)
    return out



# revision 38
# speedup vs baseline: 1.0012x; 1.0012x over previous
"""BertSelfAttention fused kernel for Trainium2, 8 NeuronCores.

Sharding: tensor-parallel over heads. 16 heads / 8 cores = 2 heads per core.
Core c owns heads 2c, 2c+1 == output feature columns [128c, 128c+128).
Every core reads the full hidden_states (pre-transposed on host to [D, B*S])
plus its 128-column slice of Wq/Wk/Wv (pre-transposed to [D, 128]); it writes
its [B*S, 128] slab of the output. No cross-core communication.

Per-core device program (B=4 batches, S=2048, D=1024, HD=64):
  stage 0: load weights/biases/mask constants; f = exp(mask) per key.
  per batch b:
    stage 1 (projection): QT,KT [128, 2048] (partition = head-elem dim,
      2 heads stacked), V [128 tok, 16 kblk, 130] where cols 0:64 head A,
      64 = denom col, 65:129 head B, 129 = denom col; V rows scaled by
      f=exp(mask) (folds the additive attention mask into the softmax
      weights exactly) and the denom cols set to f, so the softmax
      denominator comes out of the PV matmul for free.
    stage 2 (attention), per 512-query group, software-pipelined:
      for each 128-key block: S^T = K^T.T @ Q^T  (PSUM, keys on
        partitions; the two heads run as concurrent PE row-tiles)
        E = exp(S^T / 8): key blocks in DVE_KB run on the Vector engine
        (int32-cast Schraudolph seed + custom quadratic bit-correction
        op, see EXP_CORR), the rest on the Scalar engine's exp LUT —
        the two engines share what is otherwise the serial exp wall.
        ctx~ += [V|f].T @ E  (PSUM accumulate, [65, 512] per head;
                              row 64 = softmax denominator)
      epilogue: PE-transpose ctx~ 128-query chunks, out = num/den (DVE),
        DMA [128, 128] (2 heads) to the output slab.
    Scheduling: scores(i+1) is emitted before exp(i)/pv(i) including
    across query-group and batch boundaries, and the next batch's
    projection chains are emitted inside the attention loop (aligned
    with the DVE key blocks' longer exp latency) so ready projection
    matmuls fill the PE bubbles where attention waits on exp.

Matmul operands are fp16 (1 PE cycle/column vs 2 for fp32r tf32 and 4
for fp32; fp16's 10 mantissa bits keep the output within ~5e-4 of the
fp32 reference; all tensors here fit fp16 range). PSUM accumulation is
always fp32.
"""

import sys

sys.path.insert(0, "/opt/trn_rl_repo")

from contextlib import ExitStack

import numpy as np

import concourse.bass as bass
import concourse.dve_ops as dve_ops
import concourse.mybir as mybir
import concourse.tile as tile
from concourse import bacc
from concourse.bass import ds
from concourse.dve_spec import (
    C0, C1, C2, C3, Bin, One, Spec, Src0, _has_src1, _spill_c3_to_src1, lower,
)
from concourse.dve_uop import AluOp, DveOpSpec
from concourse.masks import make_identity

B, S, D = 4, 2048, 1024
H, HD = 16, 64
NCORES = 8
CW = 128  # output columns per core (2 heads * 64)
P = 128

FP32 = mybir.dt.float32
FP32R = mybir.dt.float32r
BF16 = mybir.dt.bfloat16
FP16 = mybir.dt.float16

# matmul-operand dtype: "fp32" (exact, 4 cyc/col), "fp32r" (tf32, 2 cyc/col),
# "bf16"/"fp16" (1 cyc/col; fp16 carries 10 mantissa bits vs bf16's 8 and all
# tensors here fit fp16 range), "mixed" (projections fp32r, attention fp16)
MM_DTYPE = "fp16"

# Per query group, key blocks whose softmax exp runs on the Vector engine
# (2-instruction Schraudolph: int32-cast seed + quadratic bit-field
# correction, ~3.5e-3 max rel err) instead of the Scalar engine's exp LUT.
# ACT's exp throughput (1 elem/lane/cycle @ 1.2 GHz, 33.5M elems/core) is
# the serial wall of the attention loop; splitting with DVE removes it.
DVE_KB = (2, 7, 12)

I32 = mybir.dt.int32
# seed: i = int32(score * (2^23*log2e/8) + 127*2^23); /8 folds the 1/sqrt(HD)
# softmax scale. bits(i) as fp32 = 2^k*(1+u) ~ exp(score/8).
SEED_S = float(np.float32(2.0**23) * np.float32(1.4426950408889634) / 8.0)
SEED_B = float(np.float32(127 * 2**23))
# correction: out = y * (Q0 + v*(Q1 + v*Q2)), v = 1+u via bit mask
EXP_Q0 = 1.4569739756811277
EXP_Q1 = -0.6941217487887653
EXP_Q2 = 0.23368320766312967
MASK_F = float(np.array([0x007FFFFF], np.int32).view(np.float32)[0])


def _exp_corr_ref(in0, in1, s0, s1, imm2):
    bits = np.ascontiguousarray(np.asarray(in0, np.float32)).view(np.int32)
    m = (np.float32(s0).view(np.int32) if np.ndim(s0) == 0 else
         np.asarray(s0, np.float32).reshape(-1, 1).view(np.int32))
    v = ((bits & m) | 0x3F800000).view(np.float32)
    c2 = np.float32(np.asarray(in1, np.float32).reshape(-1, 1))
    p = np.float32(s1) + v * (np.float32(imm2) + v * c2)
    return (np.asarray(in0, np.float32) * p).astype(np.float32)


def _make_exp_corr_op():
    name = "EXP_CORR_ANT"
    for o in dve_ops.OPS:
        if o.name == name:
            return o
    u = Bin(AluOp.BITWISE_AND, Src0, C0)   # C0 = mantissa mask 0x007FFFFF
    v = Bin(AluOp.BITWISE_OR, u, One)      # 1+u in [1,2)
    body = _spill_c3_to_src1(Src0 * (C1 + v * (C2 + v * C3)))
    spec = Spec(body=body, reference=_exp_corr_ref)
    row = dve_ops._CUSTOM_DVE_ROW_BASE + len(dve_ops.OPS)
    shas = {}
    for ver in ("v3", "v4"):
        shas[ver] = DveOpSpec(
            name=name, opcode=row, uops=lower(spec, ver=ver),
            rd1_en=_has_src1(spec)).sha(ver)
    op = dve_ops.DveOp(name, spec, subdim=False, uops_sha=shas)
    dve_ops.OPS.append(op)
    dve_ops.CUSTOM_DVE_SPECS[name] = spec
    dve_ops._SUB_OPCODE_FOR_NAME[name] = row
    return op


EXP_CORR = _make_exp_corr_op()


def _add_scale_ref(in0, in1, s0, s1, imm2):
    s = np.asarray(s0, np.float32).reshape(-1, 1) if np.ndim(s0) else s0
    return ((np.asarray(in0, np.float32) + np.asarray(in1, np.float32))
            * np.float32(s)).astype(np.float32)


def _make_add_scale_op():
    name = "ADD_SCALE_ANT"
    for o in dve_ops.OPS:
        if o.name == name:
            return o
    from concourse.dve_spec import Src1
    spec = Spec(body=(Src0 + Src1) * C0, reference=_add_scale_ref)
    row = dve_ops._CUSTOM_DVE_ROW_BASE + len(dve_ops.OPS)
    shas = {}
    for ver in ("v3", "v4"):
        shas[ver] = DveOpSpec(
            name=name, opcode=row, uops=lower(spec, ver=ver),
            rd1_en=_has_src1(spec)).sha(ver)
    op = dve_ops.DveOp(name, spec, subdim=False, uops_sha=shas)
    dve_ops.OPS.append(op)
    dve_ops.CUSTOM_DVE_SPECS[name] = spec
    dve_ops._SUB_OPCODE_FOR_NAME[name] = row
    return op


ADD_SCALE = _make_add_scale_op()


def _mm_dts(mm_dtype):
    """-> (projection operand dtype, attention operand dtype)"""
    if mm_dtype == "mixed":
        return FP32R, FP16
    dt = {"fp32": FP32, "fp32r": FP32R, "bf16": BF16, "fp16": FP16}[mm_dtype]
    return dt, dt


def emit_kernel(ctx: ExitStack, tc: tile.TileContext, aps: dict, b_sz: int,
                s_sz: int, mm_dtype: str):
    nc = tc.nc
    n_tok = b_sz * s_sz
    TB = min(512, s_sz)           # projection token-block / query-group size
    n_tb = s_sz // TB             # token blocks per batch
    n_kb = s_sz // P              # key blocks per batch
    n_qg = s_sz // TB             # query groups per batch
    DCH = D // P                  # contraction chunks (8)
    n_bk = b_sz * n_kb            # total key blocks

    PJ, AT = _mm_dts(mm_dtype)
    hid_t, wqt, wkt, wvt, bq, bk, bv, mask, out = (
        aps["hidden_t"], aps["wqt"], aps["wkt"], aps["wvt"], aps["bq"],
        aps["bk"], aps["bv"], aps["mask"], aps["out"])

    const = ctx.enter_context(tc.tile_pool(name="const", bufs=1))
    hidp = ctx.enter_context(tc.tile_pool(name="hidp", bufs=4))
    qkv = ctx.enter_context(tc.tile_pool(name="qkv", bufs=4))
    epool = ctx.enter_context(tc.tile_pool(name="epool", bufs=6))
    i32p = ctx.enter_context(tc.tile_pool(name="i32p", bufs=2))
    csb = ctx.enter_context(tc.tile_pool(name="csb", bufs=3))
    ostage = ctx.enter_context(tc.tile_pool(name="ostage", bufs=4))
    small = ctx.enter_context(tc.tile_pool(name="small", bufs=8))
    vtmpp = ctx.enter_context(tc.tile_pool(name="vtmpp", bufs=2))
    psA = ctx.enter_context(tc.tile_pool(name="psA", bufs=2, space="PSUM"))
    psC = ctx.enter_context(tc.tile_pool(name="psC", bufs=1, space="PSUM"))
    psP = ctx.enter_context(tc.tile_pool(name="psP", bufs=2, space="PSUM"))

    # ---- stage 0: constants ----
    # Emission order tuned for startup: the first projection chain needs
    # wq + hid(0), so those DMAs go first on the queue; the identity (which
    # gates PE's first instruction, the mask transpose) is built on gpsimd
    # before the bv-broadcast DMA is queued there.
    wq_sb = const.tile([P, DCH, CW], PJ)
    nc.sync.dma_start(wq_sb, wqt.rearrange("(c p) m -> p c m", p=P))
    bq_sb = const.tile([P, 1], FP32)
    nc.sync.dma_start(bq_sb, bq.rearrange("(p o) -> p o", o=1))
    mask_bo = const.tile([n_bk, P], FP32)
    nc.sync.dma_start(mask_bo, mask.rearrange("b (o p) -> (b o) p", p=P))

    # [P,1] tile carrying the spilled quadratic coefficient for EXP_CORR
    q2t = const.tile([P, 1], FP32)
    nc.vector.memset(q2t, EXP_Q2)

    ident = const.tile([P, P], FP32)
    f_sb = const.tile([P, n_bk], FP32)

    def emit_mask_setup():
        # Emitted after the first projection token-block so the identity
        # build (gpsimd) and mask DMA/transpose don't gate the PE's first
        # projection matmuls. f = exp(mask) is first consumed by the DVE
        # V-scaling, long after the first proj chain.
        make_identity(nc, ident)
        mask_ps = psP.tile([P, n_bk], FP32, tag="proj", name="mask_ps")
        nc.tensor.matmul(mask_ps, mask_bo, ident[:n_bk, :n_bk],
                         is_transpose=True)
        nc.scalar.activation(f_sb, mask_ps, mybir.ActivationFunctionType.Exp)

    wk_sb = const.tile([P, DCH, CW], PJ)
    nc.sync.dma_start(wk_sb, wkt.rearrange("(c p) m -> p c m", p=P))
    wv_sb = const.tile([P, DCH, CW], PJ)
    nc.sync.dma_start(wv_sb, wvt.rearrange("(c p) m -> p c m", p=P))
    bk_sb = const.tile([P, 1], FP32)
    nc.sync.dma_start(bk_sb, bk.rearrange("(p o) -> p o", o=1))
    # bv broadcast to all partitions: [128, 128], every row = bv
    bvb = const.tile([P, CW], FP32)
    nc.gpsimd.dma_start(
        out=bvb,
        in_=bass.AP(tensor=bv.tensor, offset=bv.offset, ap=[[0, P], bv.ap[0]]),
    )

    qkv_tiles: dict = {}
    hid_tiles: dict = {}

    def emit_proj_dma(b, tb):
        """Allocate batch tiles + issue the hidden-state DMA for one token
        block; the matmul chains follow via emit_proj_part."""
        if tb == 0:
            qkv_tiles[b] = (
                qkv.tile([P, s_sz], AT, tag="qt", name="qt_b"),
                qkv.tile([P, s_sz], AT, tag="kt", name="kt_b"),
                qkv.tile([P, n_kb, 130], AT, tag="v", name="v_b"),
            )
        tok0 = b * s_sz + tb * TB
        hid_tile = hidp.tile([P, DCH, TB], PJ, tag="hid", name="hid_tile")
        hid_src = hid_t.rearrange("(c p) n -> p c n", p=P)[:, :, ds(tok0, TB)]
        nc.sync.dma_start(hid_tile[:, 0:DCH // 2], hid_src[:, 0:DCH // 2])
        nc.sync.dma_start(hid_tile[:, DCH // 2:DCH],
                          hid_src[:, DCH // 2:DCH])
        hid_tiles[(b, tb)] = hid_tile

    def emit_proj_part(b, tb, part):
        """One self-contained projection matmul chain (~0.5-2.1us of PE
        work). Scattered between attention kb iterations so these
        ready-to-run matmuls fill the PE bubbles where attention waits on
        exp results."""
        qt_b, kt_b, v_b = qkv_tiles[b]
        hid_tile = hid_tiles[(b, tb)]
        if part in ("pq", "pk"):
            w_sb, bias, dst = ((wq_sb, bq_sb, qt_b) if part == "pq" else
                               (wk_sb, bk_sb, kt_b))
            ps = psP.tile([P, TB], FP32, tag="proj", name="ps")
            for c in range(DCH):
                nc.tensor.matmul(ps, w_sb[:, c, :],
                                 hid_tile[:, c, :],
                                 start=(c == 0), stop=(c == DCH - 1))
            nc.vector.tensor_scalar_add(dst[:, ds(tb * TB, TB)], ps, bias)
            return
        s4_range = (0, 1) if part == "pv01" else (2, 3)
        for s4 in s4_range:
            kbg = tb * (TB // P) + s4  # key block index within batch
            pv = psP.tile([P, CW], FP32, tag="proj", name="pv")
            for c in range(DCH):
                nc.tensor.matmul(
                    pv, hid_tile[:, c, ds(s4 * P, P)],
                    wv_sb[:, c, :],
                    start=(c == 0), stop=(c == DCH - 1))
            vtmp = vtmpp.tile([P, CW], FP32, tag="vtmp", name="vtmp")
            nc.vector.tensor_add(vtmp, pv, bvb)
            fcol = f_sb[:, ds(b * n_kb + kbg, 1)]
            nc.vector.tensor_scalar_mul(v_b[:, kbg, 0:HD], vtmp[:, 0:HD],
                                        fcol)
            nc.vector.tensor_scalar_mul(v_b[:, kbg, 65:129],
                                        vtmp[:, HD:CW], fcol)
            nc.vector.tensor_copy(v_b[:, kbg, ds(HD, 1)], fcol)
            nc.vector.tensor_copy(v_b[:, kbg, ds(129, 1)], fcol)

    PROJ_PARTS = ("pv01", "pv23", "pq", "pk")

    emit_mask_setup()
    for tb in range(n_tb):
        emit_proj_dma(0, tb)
        for part in PROJ_PARTS:
            emit_proj_part(0, tb, part)

    def emit_scores(b, qg, kb):
        qt_b, kt_b, _ = qkv_tiles[b]
        q0 = qg * TB
        st = psA.tile([P, 2 * TB], FP32, tag="st", name="st")
        nc.tensor.matmul(st[:, 0:TB],
                         kt_b[0:HD, ds(kb * P, P)],
                         qt_b[0:HD, ds(q0, TB)],
                         start=True, stop=True)
        nc.tensor.matmul(st[:, ds(TB, TB)],
                         kt_b[HD:P, ds(kb * P, P)],
                         qt_b[HD:P, ds(q0, TB)],
                         start=True, stop=True)
        return st

    # ---- stage 2: attention, software-pipelined ----
    # scores(i+1) is emitted before exp(i) and pv(i), including across
    # query-group and batch boundaries, so the PE always has the next
    # scores pair queued while exp runs. exp of each key block goes to ACT
    # (LUT) or DVE (Schraudolph seed + quadratic correction) per DVE_KB so
    # the two engines share the softmax exp wall; projection chains for the
    # next batch are emitted between exp and pv of the DVE key blocks,
    # where the PE otherwise idles for the (longer) DVE exp latency.
    groups = [(b, qg) for b in range(b_sz) for qg in range(n_qg)]
    PROJ_AT = (2, 7, 12, 14)
    st_cur = emit_scores(0, 0, 0)
    for gi, (b, qg) in enumerate(groups):
        q0 = qg * TB
        _, _, v_b = qkv_tiles[b]
        ctx_ps = psC.tile([P, 2 * TB], FP32, tag="ctx", name="ctx_ps")

        for kb in range(n_kb):
            if kb + 1 < n_kb:
                st_next = emit_scores(b, qg, kb + 1)
            elif gi + 1 < len(groups):
                st_next = emit_scores(*groups[gi + 1], 0)
            else:
                st_next = None
            if kb == 0 and b + 1 < b_sz:
                emit_proj_dma(b + 1, qg)
            e_t = epool.tile([P, 2 * TB], AT, tag="e", name="e_t")
            if kb in DVE_KB:
                i32 = i32p.tile([P, 2 * TB], I32, tag="i32", name="i32")
                for h in range(2):
                    sl = ds(h * TB, TB)
                    nc.vector.tensor_scalar(i32[:, sl], st_cur[:, sl],
                                            SEED_S, SEED_B,
                                            mybir.AluOpType.mult,
                                            mybir.AluOpType.add)
                    nc.vector._custom_dve(EXP_CORR, out=e_t[:, sl],
                                          in0=i32[:, sl].bitcast(FP32),
                                          in1=q2t, s0=MASK_F, s1=EXP_Q0,
                                          imm2=EXP_Q1)
            else:
                nc.scalar.activation(e_t, st_cur,
                                     mybir.ActivationFunctionType.Exp,
                                     scale=1.0 / 8.0)
            if kb in PROJ_AT and b + 1 < b_sz:
                emit_proj_part(b + 1, qg, PROJ_PARTS[PROJ_AT.index(kb)])
            nc.tensor.matmul(ctx_ps[0:65, 0:TB],
                             v_b[:, kb, 0:65],
                             e_t[:, 0:TB],
                             start=(kb == 0), stop=(kb == n_kb - 1))
            nc.tensor.matmul(ctx_ps[0:65, ds(TB, TB)],
                             v_b[:, kb, ds(65, 65)],
                             e_t[:, ds(TB, TB)],
                             start=(kb == 0), stop=(kb == n_kb - 1))
            st_cur = st_next

        ctx_sbs = [csb.tile([65, TB], FP32, tag="csb", name="ctx_sb")
                   for _ in range(2)]
        for h4 in range(2):
            for j in range(2):
                nc.vector.tensor_copy(
                    ctx_sbs[j][:, ds(h4 * (TB // 2), TB // 2)],
                    ctx_ps[0:65, ds(j * TB + h4 * (TB // 2), TB // 2)])
        for sub in range(TB // P):
            ost = ostage.tile([P, CW], FP32, tag="ost", name="ost")
            for j in range(2):
                tp = psP.tile([P, 65], FP32, tag="proj", name="tp")
                nc.tensor.matmul(tp, ctx_sbs[j][:, ds(sub * P, P)],
                                 ident[0:65, 0:65], is_transpose=True)
                rcp = small.tile([P, 1], FP32, tag="rcp", name="rcp")
                nc.vector.reciprocal_approx_fast(rcp, tp[:, ds(HD, 1)])
                nc.vector.tensor_scalar_mul(ost[:, ds(j * HD, HD)],
                                            tp[:, 0:HD], rcp)
            tok0 = b * s_sz + q0 + sub * P
            nc.sync.dma_start(out[ds(tok0, P), :], ost)


def build_program(b_sz=B, s_sz=S, mm_dtype=None):
    if mm_dtype is None:
        mm_dtype = MM_DTYPE
    PJ, _ = _mm_dts(mm_dtype)
    nc = bacc.Bacc("TRN2", target_bir_lowering=False, debug=False)
    n_tok = b_sz * s_sz
    aps = {
        "hidden_t": nc.dram_tensor("hidden_t", [D, n_tok], PJ,
                                   kind="ExternalInput").ap(),
        "wqt": nc.dram_tensor("wqt", [D, CW], PJ, kind="ExternalInput").ap(),
        "wkt": nc.dram_tensor("wkt", [D, CW], PJ, kind="ExternalInput").ap(),
        "wvt": nc.dram_tensor("wvt", [D, CW], PJ, kind="ExternalInput").ap(),
        "bq": nc.dram_tensor("bq", [CW], FP32, kind="ExternalInput").ap(),
        "bk": nc.dram_tensor("bk", [CW], FP32, kind="ExternalInput").ap(),
        "bv": nc.dram_tensor("bv", [CW], FP32, kind="ExternalInput").ap(),
        "mask": nc.dram_tensor("mask", [b_sz, s_sz], FP32,
                               kind="ExternalInput").ap(),
        "out": nc.dram_tensor("out", [n_tok, CW], FP32,
                              kind="ExternalOutput").ap(),
    }
    with tile.TileContext(nc) as tc:
        with ExitStack() as ctx:
            emit_kernel(ctx, tc, aps, b_sz, s_sz, mm_dtype)
    nc.compile()
    return nc


def make_in_maps(hidden_states, attention_mask, Wq, bq, Wk, bk, Wv, bv,
                 b_sz=B, s_sz=S):
    hidden_states = np.asarray(hidden_states, dtype=np.float32)
    x = hidden_states.reshape(b_sz * s_sz, D)
    hid_t = np.ascontiguousarray(x.T)
    mask = np.ascontiguousarray(
        np.broadcast_to(
            np.asarray(attention_mask, dtype=np.float32).reshape(b_sz, 1, 1,
                                                                 s_sz),
            (b_sz, 1, 1, s_sz)).reshape(b_sz, s_sz))
    Wq, Wk, Wv = (np.asarray(w, dtype=np.float32) for w in (Wq, Wk, Wv))
    bq, bk, bv = (np.asarray(v, dtype=np.float32) for v in (bq, bk, bv))
    # hidden/weights are pre-cast on the host to the projection matmul dtype
    # (same round-to-nearest the device casting DMA would apply)
    pj_np = mybir.dt.np(_mm_dts(MM_DTYPE)[0])
    hid_t = hid_t.astype(pj_np)
    in_maps = []
    for c in range(NCORES):
        rows = slice(c * CW, (c + 1) * CW)
        in_maps.append({
            "hidden_t": hid_t,
            "wqt": np.ascontiguousarray(Wq[rows, :].T).astype(pj_np),
            "wkt": np.ascontiguousarray(Wk[rows, :].T).astype(pj_np),
            "wvt": np.ascontiguousarray(Wv[rows, :].T).astype(pj_np),
            "bq": np.ascontiguousarray(bq[rows]),
            "bk": np.ascontiguousarray(bk[rows]),
            "bv": np.ascontiguousarray(bv[rows]),
            "mask": mask,
        })
    return in_maps


_NC_CACHE = {}


def _get_program():
    key = (B, S, MM_DTYPE)
    if key not in _NC_CACHE:
        _NC_CACHE[key] = build_program(B, S, MM_DTYPE)
    return _NC_CACHE[key]


def kernel(hidden_states, attention_mask, Wq, bq, Wk, bk, Wv, bv):
    from concourse.bass_utils import run_bass_kernel_spmd

    nc = _get_program()
    in_maps = make_in_maps(hidden_states, attention_mask, Wq, bq, Wk, bk, Wv,
                           bv)
    res = run_bass_kernel_spmd(nc, in_maps, list(range(NCORES)))
    out = np.empty((B, S, D), dtype=np.float32)
    for c in range(NCORES):
        out[:, :, c * CW:(c + 1) * CW] = res.results[c]["out"].reshape(
            B, S, CW)
    return out



# revision 39
# speedup vs baseline: 1.0053x; 1.0041x over previous
"""BertSelfAttention fused kernel for Trainium2, 8 NeuronCores.

Sharding: tensor-parallel over heads. 16 heads / 8 cores = 2 heads per core.
Core c owns heads 2c, 2c+1 == output feature columns [128c, 128c+128).
Every core reads the full hidden_states (pre-transposed on host to [D, B*S])
plus its 128-column slice of Wq/Wk/Wv (pre-transposed to [D, 128]); it writes
its [B*S, 128] slab of the output. No cross-core communication.

Per-core device program (B=4 batches, S=2048, D=1024, HD=64):
  stage 0: load weights/biases/mask constants; f = exp(mask) per key.
  per batch b:
    stage 1 (projection): QT,KT [128, 2048] (partition = head-elem dim,
      2 heads stacked), V [128 tok, 16 kblk, 130] where cols 0:64 head A,
      64 = denom col, 65:129 head B, 129 = denom col; V rows scaled by
      f=exp(mask) (folds the additive attention mask into the softmax
      weights exactly) and the denom cols set to f, so the softmax
      denominator comes out of the PV matmul for free.
    stage 2 (attention), per 512-query group, software-pipelined:
      for each 128-key block: S^T = K^T.T @ Q^T  (PSUM, keys on
        partitions; the two heads run as concurrent PE row-tiles)
        E = exp(S^T / 8): key blocks in DVE_KB run on the Vector engine
        (int32-cast Schraudolph seed + custom quadratic bit-correction
        op, see EXP_CORR), the rest on the Scalar engine's exp LUT —
        the two engines share what is otherwise the serial exp wall.
        ctx~ += [V|f].T @ E  (PSUM accumulate, [65, 512] per head;
                              row 64 = softmax denominator)
      epilogue: PE-transpose ctx~ 128-query chunks, out = num/den (DVE),
        DMA [128, 128] (2 heads) to the output slab.
    Scheduling: scores(i+1) is emitted before exp(i)/pv(i) including
    across query-group and batch boundaries, and the next batch's
    projection chains are emitted inside the attention loop (aligned
    with the DVE key blocks' longer exp latency) so ready projection
    matmuls fill the PE bubbles where attention waits on exp.

Matmul operands are fp16 (1 PE cycle/column vs 2 for fp32r tf32 and 4
for fp32; fp16's 10 mantissa bits keep the output within ~5e-4 of the
fp32 reference; all tensors here fit fp16 range). PSUM accumulation is
always fp32.
"""

import sys

sys.path.insert(0, "/opt/trn_rl_repo")

from contextlib import ExitStack

import numpy as np

import concourse.bass as bass
import concourse.dve_ops as dve_ops
import concourse.mybir as mybir
import concourse.tile as tile
from concourse import bacc
from concourse.bass import ds
from concourse.dve_spec import (
    C0, C1, C2, C3, Bin, One, Spec, Src0, _has_src1, _spill_c3_to_src1, lower,
)
from concourse.dve_uop import AluOp, DveOpSpec
from concourse.masks import make_identity

B, S, D = 4, 2048, 1024
H, HD = 16, 64
NCORES = 8
CW = 128  # output columns per core (2 heads * 64)
P = 128

FP32 = mybir.dt.float32
FP32R = mybir.dt.float32r
BF16 = mybir.dt.bfloat16
FP16 = mybir.dt.float16

# matmul-operand dtype: "fp32" (exact, 4 cyc/col), "fp32r" (tf32, 2 cyc/col),
# "bf16"/"fp16" (1 cyc/col; fp16 carries 10 mantissa bits vs bf16's 8 and all
# tensors here fit fp16 range), "mixed" (projections fp32r, attention fp16)
MM_DTYPE = "fp16"

# Per query group, key blocks whose softmax exp runs on the Vector engine
# (2-instruction Schraudolph: int32-cast seed + quadratic bit-field
# correction, ~3.5e-3 max rel err) instead of the Scalar engine's exp LUT.
# ACT's exp throughput (1 elem/lane/cycle @ 1.2 GHz, 33.5M elems/core) is
# the serial wall of the attention loop; splitting with DVE removes it.
DVE_KB = (2, 7, 12)

I32 = mybir.dt.int32
# seed: i = int32(score * (2^23*log2e/8) + 127*2^23); /8 folds the 1/sqrt(HD)
# softmax scale. bits(i) as fp32 = 2^k*(1+u) ~ exp(score/8).
SEED_S = float(np.float32(2.0**23) * np.float32(1.4426950408889634) / 8.0)
SEED_B = float(np.float32(127 * 2**23))
# correction: out = y * (Q0 + v*(Q1 + v*Q2)), v = 1+u via bit mask
EXP_Q0 = 1.4569739756811277
EXP_Q1 = -0.6941217487887653
EXP_Q2 = 0.23368320766312967
MASK_F = float(np.array([0x007FFFFF], np.int32).view(np.float32)[0])


def _exp_corr_ref(in0, in1, s0, s1, imm2):
    bits = np.ascontiguousarray(np.asarray(in0, np.float32)).view(np.int32)
    m = (np.float32(s0).view(np.int32) if np.ndim(s0) == 0 else
         np.asarray(s0, np.float32).reshape(-1, 1).view(np.int32))
    v = ((bits & m) | 0x3F800000).view(np.float32)
    c2 = np.float32(np.asarray(in1, np.float32).reshape(-1, 1))
    p = np.float32(s1) + v * (np.float32(imm2) + v * c2)
    return (np.asarray(in0, np.float32) * p).astype(np.float32)


def _make_exp_corr_op():
    name = "EXP_CORR_ANT"
    for o in dve_ops.OPS:
        if o.name == name:
            return o
    u = Bin(AluOp.BITWISE_AND, Src0, C0)   # C0 = mantissa mask 0x007FFFFF
    v = Bin(AluOp.BITWISE_OR, u, One)      # 1+u in [1,2)
    body = _spill_c3_to_src1(Src0 * (C1 + v * (C2 + v * C3)))
    spec = Spec(body=body, reference=_exp_corr_ref)
    row = dve_ops._CUSTOM_DVE_ROW_BASE + len(dve_ops.OPS)
    shas = {}
    for ver in ("v3", "v4"):
        shas[ver] = DveOpSpec(
            name=name, opcode=row, uops=lower(spec, ver=ver),
            rd1_en=_has_src1(spec)).sha(ver)
    op = dve_ops.DveOp(name, spec, subdim=False, uops_sha=shas)
    dve_ops.OPS.append(op)
    dve_ops.CUSTOM_DVE_SPECS[name] = spec
    dve_ops._SUB_OPCODE_FOR_NAME[name] = row
    return op


EXP_CORR = _make_exp_corr_op()


def _add_scale_ref(in0, in1, s0, s1, imm2):
    s = np.asarray(s0, np.float32).reshape(-1, 1) if np.ndim(s0) else s0
    return ((np.asarray(in0, np.float32) + np.asarray(in1, np.float32))
            * np.float32(s)).astype(np.float32)


def _make_add_scale_op():
    name = "ADD_SCALE_ANT"
    for o in dve_ops.OPS:
        if o.name == name:
            return o
    from concourse.dve_spec import Src1
    spec = Spec(body=(Src0 + Src1) * C0, reference=_add_scale_ref)
    row = dve_ops._CUSTOM_DVE_ROW_BASE + len(dve_ops.OPS)
    shas = {}
    for ver in ("v3", "v4"):
        shas[ver] = DveOpSpec(
            name=name, opcode=row, uops=lower(spec, ver=ver),
            rd1_en=_has_src1(spec)).sha(ver)
    op = dve_ops.DveOp(name, spec, subdim=False, uops_sha=shas)
    dve_ops.OPS.append(op)
    dve_ops.CUSTOM_DVE_SPECS[name] = spec
    dve_ops._SUB_OPCODE_FOR_NAME[name] = row
    return op


ADD_SCALE = _make_add_scale_op()


def _mm_dts(mm_dtype):
    """-> (projection operand dtype, attention operand dtype)"""
    if mm_dtype == "mixed":
        return FP32R, FP16
    dt = {"fp32": FP32, "fp32r": FP32R, "bf16": BF16, "fp16": FP16}[mm_dtype]
    return dt, dt


def emit_kernel(ctx: ExitStack, tc: tile.TileContext, aps: dict, b_sz: int,
                s_sz: int, mm_dtype: str):
    nc = tc.nc
    n_tok = b_sz * s_sz
    TB = min(512, s_sz)           # projection token-block / query-group size
    n_tb = s_sz // TB             # token blocks per batch
    n_kb = s_sz // P              # key blocks per batch
    n_qg = s_sz // TB             # query groups per batch
    DCH = D // P                  # contraction chunks (8)
    n_bk = b_sz * n_kb            # total key blocks

    PJ, AT = _mm_dts(mm_dtype)
    hid_t, wqt, wkt, wvt, bq, bk, bv, mask, out = (
        aps["hidden_t"], aps["wqt"], aps["wkt"], aps["wvt"], aps["bq"],
        aps["bk"], aps["bv"], aps["mask"], aps["out"])

    const = ctx.enter_context(tc.tile_pool(name="const", bufs=1))
    hidp = ctx.enter_context(tc.tile_pool(name="hidp", bufs=4))
    qkv = ctx.enter_context(tc.tile_pool(name="qkv", bufs=4))
    epool = ctx.enter_context(tc.tile_pool(name="epool", bufs=6))
    i32p = ctx.enter_context(tc.tile_pool(name="i32p", bufs=2))
    csb = ctx.enter_context(tc.tile_pool(name="csb", bufs=3))
    ostage = ctx.enter_context(tc.tile_pool(name="ostage", bufs=4))
    small = ctx.enter_context(tc.tile_pool(name="small", bufs=8))
    vtmpp = ctx.enter_context(tc.tile_pool(name="vtmpp", bufs=2))
    psA = ctx.enter_context(tc.tile_pool(name="psA", bufs=2, space="PSUM"))
    psC = ctx.enter_context(tc.tile_pool(name="psC", bufs=1, space="PSUM"))
    psP = ctx.enter_context(tc.tile_pool(name="psP", bufs=2, space="PSUM"))

    # ---- stage 0: constants ----
    # Emission order tuned for startup: the first projection chain needs
    # wq + hid(0), so those DMAs go first on the queue; the identity (which
    # gates PE's first instruction, the mask transpose) is built on gpsimd
    # before the bv-broadcast DMA is queued there.
    wq_sb = const.tile([P, DCH, CW], PJ)
    nc.sync.dma_start(wq_sb, wqt.rearrange("(c p) m -> p c m", p=P))
    bq_sb = const.tile([P, 1], FP32)
    nc.sync.dma_start(bq_sb, bq.rearrange("(p o) -> p o", o=1))
    mask_bo = const.tile([n_bk, P], FP32)
    nc.sync.dma_start(mask_bo, mask.rearrange("b (o p) -> (b o) p", p=P))

    # [P,1] tile carrying the spilled quadratic coefficient for EXP_CORR
    q2t = const.tile([P, 1], FP32)
    nc.vector.memset(q2t, EXP_Q2)

    ident = const.tile([P, P], FP32)
    f_sb = const.tile([P, n_bk], FP32)

    def emit_mask_setup():
        # Emitted after the first projection token-block so the identity
        # build (gpsimd) and mask DMA/transpose don't gate the PE's first
        # projection matmuls. f = exp(mask) is first consumed by the DVE
        # V-scaling, long after the first proj chain.
        make_identity(nc, ident)
        mask_ps = psP.tile([P, n_bk], FP32, tag="proj", name="mask_ps")
        nc.tensor.matmul(mask_ps, mask_bo, ident[:n_bk, :n_bk],
                         is_transpose=True)
        nc.scalar.activation(f_sb, mask_ps, mybir.ActivationFunctionType.Exp)

    wk_sb = const.tile([P, DCH, CW], PJ)
    nc.sync.dma_start(wk_sb, wkt.rearrange("(c p) m -> p c m", p=P))
    wv_sb = const.tile([P, DCH, CW], PJ)
    nc.sync.dma_start(wv_sb, wvt.rearrange("(c p) m -> p c m", p=P))
    bk_sb = const.tile([P, 1], FP32)
    nc.sync.dma_start(bk_sb, bk.rearrange("(p o) -> p o", o=1))
    # bv broadcast to all partitions: [128, 128], every row = bv
    bvb = const.tile([P, CW], FP32)
    nc.gpsimd.dma_start(
        out=bvb,
        in_=bass.AP(tensor=bv.tensor, offset=bv.offset, ap=[[0, P], bv.ap[0]]),
    )

    qkv_tiles: dict = {}
    hid_tiles: dict = {}

    def emit_proj_dma(b, tb):
        """Allocate batch tiles + issue the hidden-state DMA for one token
        block; the matmul chains follow via emit_proj_part."""
        if tb == 0:
            qkv_tiles[b] = (
                qkv.tile([P, s_sz], AT, tag="qt", name="qt_b"),
                qkv.tile([P, s_sz], AT, tag="kt", name="kt_b"),
                qkv.tile([P, n_kb, 130], AT, tag="v", name="v_b"),
            )
        tok0 = b * s_sz + tb * TB
        hid_tile = hidp.tile([P, DCH, TB], PJ, tag="hid", name="hid_tile")
        hid_src = hid_t.rearrange("(c p) n -> p c n", p=P)[:, :, ds(tok0, TB)]
        nc.sync.dma_start(hid_tile[:, 0:DCH // 2], hid_src[:, 0:DCH // 2])
        nc.sync.dma_start(hid_tile[:, DCH // 2:DCH],
                          hid_src[:, DCH // 2:DCH])
        hid_tiles[(b, tb)] = hid_tile

    def emit_proj_part(b, tb, part):
        """One self-contained projection matmul chain (~0.5-2.1us of PE
        work). Scattered between attention kb iterations so these
        ready-to-run matmuls fill the PE bubbles where attention waits on
        exp results."""
        qt_b, kt_b, v_b = qkv_tiles[b]
        hid_tile = hid_tiles[(b, tb)]
        if part in ("pq", "pk"):
            w_sb, bias, dst = ((wq_sb, bq_sb, qt_b) if part == "pq" else
                               (wk_sb, bk_sb, kt_b))
            ps = psP.tile([P, TB], FP32, tag="proj", name="ps")
            for c in range(DCH):
                nc.tensor.matmul(ps, w_sb[:, c, :],
                                 hid_tile[:, c, :],
                                 start=(c == 0), stop=(c == DCH - 1))
            nc.vector.tensor_scalar_add(dst[:, ds(tb * TB, TB)], ps, bias)
            return
        s4_range = (0, 1) if part == "pv01" else (2, 3)
        for s4 in s4_range:
            kbg = tb * (TB // P) + s4  # key block index within batch
            pv = psP.tile([P, CW], FP32, tag="proj", name="pv")
            for c in range(DCH):
                nc.tensor.matmul(
                    pv, hid_tile[:, c, ds(s4 * P, P)],
                    wv_sb[:, c, :],
                    start=(c == 0), stop=(c == DCH - 1))
            vtmp = vtmpp.tile([P, CW], FP32, tag="vtmp", name="vtmp")
            nc.vector.tensor_add(vtmp, pv, bvb)
            fcol = f_sb[:, ds(b * n_kb + kbg, 1)]
            nc.vector.tensor_scalar_mul(v_b[:, kbg, 0:HD], vtmp[:, 0:HD],
                                        fcol)
            nc.vector.tensor_scalar_mul(v_b[:, kbg, 65:129],
                                        vtmp[:, HD:CW], fcol)
            nc.vector.tensor_copy(v_b[:, kbg, ds(HD, 1)], fcol)
            nc.vector.tensor_copy(v_b[:, kbg, ds(129, 1)], fcol)

    PROJ_PARTS = ("pv01", "pv23", "pq", "pk")

    emit_mask_setup()
    for tb in range(n_tb):
        emit_proj_dma(0, tb)
        for part in PROJ_PARTS:
            emit_proj_part(0, tb, part)

    def emit_scores(b, qg, kb):
        qt_b, kt_b, _ = qkv_tiles[b]
        q0 = qg * TB
        st = psA.tile([P, 2 * TB], FP32, tag="st", name="st")
        nc.tensor.matmul(st[:, 0:TB],
                         kt_b[0:HD, ds(kb * P, P)],
                         qt_b[0:HD, ds(q0, TB)],
                         start=True, stop=True)
        nc.tensor.matmul(st[:, ds(TB, TB)],
                         kt_b[HD:P, ds(kb * P, P)],
                         qt_b[HD:P, ds(q0, TB)],
                         start=True, stop=True)
        return st

    # ---- stage 2: attention, software-pipelined ----
    # scores(i+1) is emitted before exp(i) and pv(i), including across
    # query-group and batch boundaries, so the PE always has the next
    # scores pair queued while exp runs. exp of each key block goes to ACT
    # (LUT) or DVE (Schraudolph seed + quadratic correction) per DVE_KB so
    # the two engines share the softmax exp wall; projection chains for the
    # next batch are emitted between exp and pv of the DVE key blocks,
    # where the PE otherwise idles for the (longer) DVE exp latency.
    groups = [(b, qg) for b in range(b_sz) for qg in range(n_qg)]
    PROJ_AT = (2, 7, 12, 14)
    st_cur = emit_scores(0, 0, 0)
    st_ahead = None   # scores(next group, kb=1), pre-emitted in the epilogue
    for gi, (b, qg) in enumerate(groups):
        q0 = qg * TB
        _, _, v_b = qkv_tiles[b]
        ctx_ps = psC.tile([P, 2 * TB], FP32, tag="ctx", name="ctx_ps")

        for kb in range(n_kb):
            if kb == 0 and st_ahead is not None:
                st_next = st_ahead
                st_ahead = None
            elif kb + 1 < n_kb:
                st_next = emit_scores(b, qg, kb + 1)
            elif gi + 1 < len(groups):
                st_next = emit_scores(*groups[gi + 1], 0)
            else:
                st_next = None
            if kb == 0 and b + 1 < b_sz:
                emit_proj_dma(b + 1, qg)
            e_t = epool.tile([P, 2 * TB], AT, tag="e", name="e_t")
            if kb in DVE_KB:
                i32 = i32p.tile([P, 2 * TB], I32, tag="i32", name="i32")
                for h in range(2):
                    sl = ds(h * TB, TB)
                    nc.vector.tensor_scalar(i32[:, sl], st_cur[:, sl],
                                            SEED_S, SEED_B,
                                            mybir.AluOpType.mult,
                                            mybir.AluOpType.add)
                    nc.vector._custom_dve(EXP_CORR, out=e_t[:, sl],
                                          in0=i32[:, sl].bitcast(FP32),
                                          in1=q2t, s0=MASK_F, s1=EXP_Q0,
                                          imm2=EXP_Q1)
            else:
                nc.scalar.activation(e_t, st_cur,
                                     mybir.ActivationFunctionType.Exp,
                                     scale=1.0 / 8.0)
            if kb in PROJ_AT and b + 1 < b_sz:
                emit_proj_part(b + 1, qg, PROJ_PARTS[PROJ_AT.index(kb)])
            nc.tensor.matmul(ctx_ps[0:65, 0:TB],
                             v_b[:, kb, 0:65],
                             e_t[:, 0:TB],
                             start=(kb == 0), stop=(kb == n_kb - 1))
            nc.tensor.matmul(ctx_ps[0:65, ds(TB, TB)],
                             v_b[:, kb, ds(65, 65)],
                             e_t[:, ds(TB, TB)],
                             start=(kb == 0), stop=(kb == n_kb - 1))
            st_cur = st_next

        ctx_sbs = [csb.tile([65, TB], FP32, tag="csb", name="ctx_sb")
                   for _ in range(2)]
        for h4 in range(2):
            for j in range(2):
                nc.vector.tensor_copy(
                    ctx_sbs[j][:, ds(h4 * (TB // 2), TB // 2)],
                    ctx_ps[0:65, ds(j * TB + h4 * (TB // 2), TB // 2)])
        if gi + 1 < len(groups):
            # pre-emit the next group's kb1 scores between PV(15) and the
            # transposes in the PE FIFO: it covers the ~300ns handoff where
            # the first transpose waits on the DVE ctx-copy chunk.
            st_ahead = emit_scores(*groups[gi + 1], 1)
        for sub in range(TB // P):
            ost = ostage.tile([P, CW], FP32, tag="ost", name="ost")
            for j in range(2):
                tp = psP.tile([P, 65], FP32, tag="proj", name="tp")
                nc.tensor.matmul(tp, ctx_sbs[j][:, ds(sub * P, P)],
                                 ident[0:65, 0:65], is_transpose=True)
                rcp = small.tile([P, 1], FP32, tag="rcp", name="rcp")
                nc.vector.reciprocal_approx_fast(rcp, tp[:, ds(HD, 1)])
                nc.vector.tensor_scalar_mul(ost[:, ds(j * HD, HD)],
                                            tp[:, 0:HD], rcp)
            tok0 = b * s_sz + q0 + sub * P
            nc.sync.dma_start(out[ds(tok0, P), :], ost)


def build_program(b_sz=B, s_sz=S, mm_dtype=None):
    if mm_dtype is None:
        mm_dtype = MM_DTYPE
    PJ, _ = _mm_dts(mm_dtype)
    nc = bacc.Bacc("TRN2", target_bir_lowering=False, debug=False)
    n_tok = b_sz * s_sz
    aps = {
        "hidden_t": nc.dram_tensor("hidden_t", [D, n_tok], PJ,
                                   kind="ExternalInput").ap(),
        "wqt": nc.dram_tensor("wqt", [D, CW], PJ, kind="ExternalInput").ap(),
        "wkt": nc.dram_tensor("wkt", [D, CW], PJ, kind="ExternalInput").ap(),
        "wvt": nc.dram_tensor("wvt", [D, CW], PJ, kind="ExternalInput").ap(),
        "bq": nc.dram_tensor("bq", [CW], FP32, kind="ExternalInput").ap(),
        "bk": nc.dram_tensor("bk", [CW], FP32, kind="ExternalInput").ap(),
        "bv": nc.dram_tensor("bv", [CW], FP32, kind="ExternalInput").ap(),
        "mask": nc.dram_tensor("mask", [b_sz, s_sz], FP32,
                               kind="ExternalInput").ap(),
        "out": nc.dram_tensor("out", [n_tok, CW], FP32,
                              kind="ExternalOutput").ap(),
    }
    with tile.TileContext(nc) as tc:
        with ExitStack() as ctx:
            emit_kernel(ctx, tc, aps, b_sz, s_sz, mm_dtype)
    nc.compile()
    return nc


def make_in_maps(hidden_states, attention_mask, Wq, bq, Wk, bk, Wv, bv,
                 b_sz=B, s_sz=S):
    hidden_states = np.asarray(hidden_states, dtype=np.float32)
    x = hidden_states.reshape(b_sz * s_sz, D)
    hid_t = np.ascontiguousarray(x.T)
    mask = np.ascontiguousarray(
        np.broadcast_to(
            np.asarray(attention_mask, dtype=np.float32).reshape(b_sz, 1, 1,
                                                                 s_sz),
            (b_sz, 1, 1, s_sz)).reshape(b_sz, s_sz))
    Wq, Wk, Wv = (np.asarray(w, dtype=np.float32) for w in (Wq, Wk, Wv))
    bq, bk, bv = (np.asarray(v, dtype=np.float32) for v in (bq, bk, bv))
    # hidden/weights are pre-cast on the host to the projection matmul dtype
    # (same round-to-nearest the device casting DMA would apply)
    pj_np = mybir.dt.np(_mm_dts(MM_DTYPE)[0])
    hid_t = hid_t.astype(pj_np)
    in_maps = []
    for c in range(NCORES):
        rows = slice(c * CW, (c + 1) * CW)
        in_maps.append({
            "hidden_t": hid_t,
            "wqt": np.ascontiguousarray(Wq[rows, :].T).astype(pj_np),
            "wkt": np.ascontiguousarray(Wk[rows, :].T).astype(pj_np),
            "wvt": np.ascontiguousarray(Wv[rows, :].T).astype(pj_np),
            "bq": np.ascontiguousarray(bq[rows]),
            "bk": np.ascontiguousarray(bk[rows]),
            "bv": np.ascontiguousarray(bv[rows]),
            "mask": mask,
        })
    return in_maps


_NC_CACHE = {}


def _get_program():
    key = (B, S, MM_DTYPE)
    if key not in _NC_CACHE:
        _NC_CACHE[key] = build_program(B, S, MM_DTYPE)
    return _NC_CACHE[key]


def kernel(hidden_states, attention_mask, Wq, bq, Wk, bk, Wv, bv):
    from concourse.bass_utils import run_bass_kernel_spmd

    nc = _get_program()
    in_maps = make_in_maps(hidden_states, attention_mask, Wq, bq, Wk, bk, Wv,
                           bv)
    res = run_bass_kernel_spmd(nc, in_maps, list(range(NCORES)))
    out = np.empty((B, S, D), dtype=np.float32)
    for c in range(NCORES):
        out[:, :, c * CW:(c + 1) * CW] = res.results[c]["out"].reshape(
            B, S, CW)
    return out

